# revision 1
# baseline (speedup 1.0000x reference)
"""Trainium2 Bass kernel for CustomMinkowskiLayerNorm.

Math (matches the jax reference):
    counts[b]  = #points with batch_indices == b           (clamped >= 1)
    mean[b,c]  = sum_{i in b} x[i,c] / counts[b]
    var[b,c]   = sum_{i in b} (x[i,c]-mean)^2 / counts[b]  (= E[x^2]-mean^2)
    out[i,c]   = (x[i,c]-mean[b_i,c]) / sqrt(var[b_i,c]+eps) * gamma[c] + beta[c]

Sharding: batch_indices is sorted and BATCH == n_cores == 8, so each core owns
exactly one batch segment -> all segment reductions are core-local, no
collectives. The host splits at segment boundaries (searchsorted), transposes
each segment to channel-major layout and zero-pads to a fixed shape:

    xt[p, f], p in [0,128): partition p < 64  = channel p,  points [0, F_HALF)
                            partition p >= 64 = channel p-64, points [F_HALF, 2*F_HALF)

Channel-major layout makes the per-channel segment reduction a free-dim
reduction and the normalization a single per-partition affine op (DVE
tensor_scalar, fp32 2x perf mode).

Device program (per core, identical SPMD):
  pass 1: DMA tiles of [128, 2048] on the sync HWDGE ring. Stats per tile:
          most tiles -> DVE bn_stats (one op per 512 chunk); N_ACT of the
          cached tiles -> ScalarE Copy/Square activations with the
          per-partition accum_out reducer (scratch output goes to PSUM),
          which keeps the DVE off the pass-1 critical path. The first
          NCACHE tiles stay resident in SBUF.
  stats:  bn_aggr (split: all-but-last tile early) + ACT accumulators ->
          raw (sum, sumsq); fold partitions p/p+64 and broadcast with one
          TensorE matmul against a 0/1 fold matrix; apply 1/count; rstd =
          1/sqrt(var+eps) with 2 Newton refinements (ACT sqrt table is
          low-precision); s = gamma*rstd, t = beta - mean*s.
  pass 2: x_tile = x_tile * s + t in place; stores on the scalar HWDGE
          ring. Non-resident tiles are re-read on the SWDGE ring (issued
          after pass 1 in program order; SWDGE loads clash with concurrent
          HWDGE loads but coexist with HWDGE stores). The small-input DMAs
          at the sync ring head delay the pass-1 burst ~8us so the SWDGE
          prefetch of the first re-reads finishes before it ramps.
"""

import os
import sys

for _p in ("/opt/trn_rl_repo", "/root/.axon_site/_ro/trn_rl_repo"):
    if os.path.isdir(_p) and _p not in sys.path:
        sys.path.append(_p)

from contextlib import ExitStack

import numpy as np

import concourse.bacc as bacc
import concourse.tile as tile
from concourse import mybir
from concourse._compat import with_exitstack
from concourse.bass_utils import run_bass_kernel_spmd

F32 = mybir.dt.float32

N = 1_000_000
C = 64
BATCH = 8
EPS = 1e-5

P = 128            # SBUF partitions
F_TILE = 2048      # free elems per tile -> [128, 2048] f32 = 1 MiB per DMA
BN_F = 512         # bn_stats free-dim max
NCACHE_MAX = 20    # tiles kept resident in SBUF between passes
LOAD_BUFS = 3      # rotating pass-1 load slots
P2_BUFS = 2        # rotating pass-2 re-read slots
N_ACT = 0          # cached tiles whose stats run on ScalarE (0: extra
                   # cross-engine sem structure triggers a Tile mid-kernel
                   # sem-rollover drain that costs more than it saves)

_mult = mybir.AluOpType.mult
_add = mybir.AluOpType.add

_AF = mybir.ActivationFunctionType


def _make_body(f_half: int):
    nt = f_half // F_TILE
    ncache = min(NCACHE_MAX, nt)
    # Per-tile stats cost: bn_stats path ~2.7us all-DVE; ACT path ~4.3us
    # all-ACT (Copy+accum, Square+accum). ~12/31 tiles on ACT balances both
    # engines under the ~2.4us/tile DMA delivery rate. Only CACHED tiles go
    # to ACT: rotating lpool slots must recycle at the faster DVE rate.
    n_act = max(0, min(ncache, N_ACT))
    act_set = set()
    for i in range(n_act):
        act_set.add(int((i + 0.5) * ncache / n_act))
    dve_tiles = [t for t in range(nt) if t not in act_set]
    n_act = len(act_set)

    @with_exitstack
    def _body(ctx: ExitStack, tc: tile.TileContext,
              out_ap, xt_ap, invn_ap, gcol_ap, bcol_ap, foldm_ap):
        nc = tc.nc

        cache = ctx.enter_context(tc.tile_pool(name="cache", bufs=ncache))
        lpool = ctx.enter_context(tc.tile_pool(name="lpool", bufs=LOAD_BUFS))
        p2pool = ctx.enter_context(tc.tile_pool(name="p2pool", bufs=P2_BUFS))
        small = ctx.enter_context(tc.tile_pool(name="small", bufs=1))
        psum = ctx.enter_context(tc.tile_pool(name="psum", bufs=1, space="PSUM"))

        ngrp = F_TILE // BN_F
        stats = small.tile([P, max(len(dve_tiles), 1) * ngrp, 6], F32,
                           tag="stats")
        accs = None
        pscratch = None
        if n_act:
            accs = small.tile([P, n_act, 2], F32, tag="accs")
            pscratch = psum.tile([P, F_TILE], F32, tag="pscratch")

        # Small inputs at the head of the sync ring: their ~8us of ring time
        # intentionally delays the pass-1 load burst (see module docstring).
        invn_sb = small.tile([P, 1], F32, tag="invn")
        gcol_sb = small.tile([P, 1], F32, tag="gcol")
        bcol_sb = small.tile([P, 1], F32, tag="bcol")
        foldm_sb = small.tile([P, P], F32, tag="foldm")
        nc.sync.dma_start(out=invn_sb, in_=invn_ap)
        nc.sync.dma_start(out=gcol_sb, in_=gcol_ap)
        nc.sync.dma_start(out=bcol_sb, in_=bcol_ap)
        nc.sync.dma_start(out=foldm_sb, in_=foldm_ap)

        # Pre-load the ACT sqrt table set so the stats chain later doesn't
        # stall on ACT_TABLE_LOAD.
        warm = small.tile([P, 1], F32, tag="warm")
        nc.vector.memset(warm, 1.0)
        nc.scalar.activation(out=warm, in_=warm, func=_AF.Sqrt)

        # ---- pass 1: stream all tiles; stats on DVE bn_stats or ACT ----
        cached = {}
        dve_grp = 0
        act_idx = 0
        for t in range(nt):
            sl = slice(t * F_TILE, (t + 1) * F_TILE)
            if t < ncache:
                xt = cache.tile([P, F_TILE], F32, tag="c")
                cached[t] = xt
            else:
                xt = lpool.tile([P, F_TILE], F32, tag="l")
            nc.sync.dma_start(out=xt, in_=xt_ap[:, sl])
            if t in act_set:
                nc.scalar.activation(out=pscratch, in_=xt, func=_AF.Copy,
                                     accum_out=accs[:, act_idx, 0:1])
                nc.scalar.activation(out=pscratch, in_=xt, func=_AF.Square,
                                     accum_out=accs[:, act_idx, 1:2])
                act_idx += 1
            else:
                for j in range(ngrp):
                    nc.vector.bn_stats(
                        out=stats[:, dve_grp, :],
                        in_=xt[:, j * BN_F : (j + 1) * BN_F],
                    )
                    dve_grp += 1

        # Pass-2 re-read DMAs on the SWDGE (gpsimd) ring, after pass 1 in
        # program order. SWDGE loads clash with concurrent HWDGE loads but
        # coexist fine with the HWDGE stores they will actually run beside.
        p2tiles = {}
        for t in range(ncache, nt):
            sl = slice(t * F_TILE, (t + 1) * F_TILE)
            xt = p2pool.tile([P, F_TILE], F32, tag="p2")
            nc.gpsimd.dma_start(out=xt, in_=xt_ap[:, sl])
            p2tiles[t] = xt

        # ---- aggregate stats ----
        # Split bn_aggr so only the last chunk aggregates on the critical
        # path after the final bn_stats.
        gtot = len(dve_tiles) * ngrp
        ga = max(gtot - ngrp, 1)
        mva = small.tile([P, 2], F32, tag="mva")
        mvb = small.tile([P, 2], F32, tag="mvb")
        nc.vector.bn_aggr(out=mva, in_=stats[:, :ga, :])
        nc.vector.bn_aggr(out=mvb, in_=stats[:, ga:, :])

        def raw_sums(dst, mv, n):
            # dst[:,0] = mean*n ; dst[:,1] = (var+mean^2)*n
            m2 = small.tile([P, 1], F32, tag="m2tmp")
            nc.vector.tensor_mul(out=m2, in0=mv[:, 0:1], in1=mv[:, 0:1])
            nc.vector.tensor_add(out=m2, in0=m2, in1=mv[:, 1:2])
            nc.vector.tensor_scalar_mul(out=dst[:, 0:1], in0=mv[:, 0:1],
                                        scalar1=float(n))
            nc.vector.tensor_scalar_mul(out=dst[:, 1:2], in0=m2,
                                        scalar1=float(n))

        sums_a = small.tile([P, 2], F32, tag="sums_a")
        sums_b = small.tile([P, 2], F32, tag="sums_b")
        raw_sums(sums_a, mva, ga * BN_F)
        raw_sums(sums_b, mvb, (gtot - ga) * BN_F)
        sums = small.tile([P, 2], F32, tag="sums")
        nc.vector.tensor_add(out=sums, in0=sums_a, in1=sums_b)
        if n_act:
            asums = small.tile([P, 2], F32, tag="asums")
            acc_view = accs.rearrange("p t c -> p c t")
            nc.vector.reduce_sum(out=asums, in_=acc_view,
                                 axis=mybir.AxisListType.X)
            nc.vector.tensor_add(out=sums, in0=sums, in1=asums)

        # ---- fold halves + broadcast: tot[p] = sums[p%64] + sums[p%64+64] ----
        ptot = psum.tile([P, 2], F32, tag="pt")
        nc.tensor.matmul(out=ptot, lhsT=foldm_sb, rhs=sums,
                         start=True, stop=True)
        tot = small.tile([P, 2], F32, tag="tot")
        nc.vector.tensor_copy(out=tot, in_=ptot)

        # ---- per-channel coefficients ----
        mm = small.tile([P, 2], F32, tag="mm")      # (mean, E[x^2])
        nc.vector.tensor_scalar_mul(out=mm, in0=tot, scalar1=invn_sb[:, 0:1])
        var = small.tile([P, 1], F32, tag="var")
        nc.vector.tensor_mul(out=var, in0=mm[:, 0:1], in1=mm[:, 0:1])
        nc.vector.tensor_sub(out=var, in0=mm[:, 1:2], in1=var)
        v = small.tile([P, 1], F32, tag="v")
        nc.vector.tensor_scalar(out=v, in0=var, scalar1=0.0, scalar2=EPS,
                                op0=mybir.AluOpType.max, op1=_add)
        r = small.tile([P, 1], F32, tag="r")
        nc.scalar.activation(out=r, in_=v, func=_AF.Sqrt)
        nc.vector.reciprocal(out=r, in_=r)
        a = small.tile([P, 1], F32, tag="a")
        for _ in range(2):
            nc.vector.tensor_mul(out=a, in0=r, in1=r)
            nc.vector.tensor_mul(out=a, in0=a, in1=v)
            nc.vector.tensor_scalar(out=a, in0=a, scalar1=-0.5, scalar2=1.5,
                                    op0=_mult, op1=_add)
            nc.vector.tensor_mul(out=r, in0=r, in1=a)
        s_col = small.tile([P, 1], F32, tag="s_col")
        nc.vector.tensor_mul(out=s_col, in0=r, in1=gcol_sb)
        t_col = small.tile([P, 1], F32, tag="t_col")
        nc.vector.tensor_mul(out=t_col, in0=mm[:, 0:1], in1=s_col)
        nc.vector.tensor_sub(out=t_col, in0=bcol_sb, in1=t_col)

        # ---- pass 2: x = x*s + t in place, store on scalar ring ----
        # Interleave non-resident tiles among resident ones so their re-read
        # slots recycle while stores stream.
        cu, uu = list(range(ncache)), list(range(ncache, nt))
        order = []
        while cu or uu:
            if uu:
                order.append(uu.pop(0))
            order.extend(cu[:2])
            del cu[:2]
        for t in order:
            sl = slice(t * F_TILE, (t + 1) * F_TILE)
            xt = cached[t] if t < ncache else p2tiles[t]
            nc.vector.tensor_scalar(out=xt, in0=xt, scalar1=s_col[:, 0:1],
                                    scalar2=t_col[:, 0:1], op0=_mult, op1=_add)
            nc.scalar.dma_start(out=out_ap[:, sl], in_=xt)

    return _body


_NC_CACHE = {}


def _build_program(f_half: int):
    if f_half in _NC_CACHE:
        return _NC_CACHE[f_half]
    nc = bacc.Bacc("TRN2", target_bir_lowering=False, debug=False,
                   num_devices=BATCH)
    xt = nc.dram_tensor("xt", [P, f_half], F32, kind="ExternalInput").ap()
    invn = nc.dram_tensor("invn", [P, 1], F32, kind="ExternalInput").ap()
    gcol = nc.dram_tensor("gcol", [P, 1], F32, kind="ExternalInput").ap()
    bcol = nc.dram_tensor("bcol", [P, 1], F32, kind="ExternalInput").ap()
    foldm = nc.dram_tensor("foldm", [P, P], F32, kind="ExternalInput").ap()
    out = nc.dram_tensor("out", [P, f_half], F32, kind="ExternalOutput").ap()
    with tile.TileContext(nc) as tc:
        _make_body(f_half)(tc, out, xt, invn, gcol, bcol, foldm)
    nc.compile()
    _NC_CACHE[f_half] = nc
    return nc


def _prepare(features, batch_indices, gamma, beta):
    features = np.asarray(features, dtype=np.float32)
    batch_indices = np.asarray(batch_indices, dtype=np.int32)
    gamma = np.asarray(gamma, dtype=np.float32)
    beta = np.asarray(beta, dtype=np.float32)

    bounds = np.searchsorted(batch_indices, np.arange(BATCH + 1), side="left")
    cnts = np.diff(bounds)
    # fixed SPMD shape: half-row length, padded to a multiple of F_TILE
    f_half = max(int(-(-int(cnts.max()) // 2 // F_TILE) * F_TILE), F_TILE)

    gcol = np.concatenate([gamma, gamma]).reshape(P, 1).astype(np.float32)
    bcol = np.concatenate([beta, beta]).reshape(P, 1).astype(np.float32)
    k = np.arange(P)
    foldm = (k[:, None] % C == k[None, :] % C).astype(np.float32)

    in_maps = []
    for b in range(BATCH):
        s, e = int(bounds[b]), int(bounds[b + 1])
        cnt = e - s
        xt = np.zeros((P, f_half), dtype=np.float32)
        n1 = min(cnt, f_half)
        if n1 > 0:
            xt[0:C, :n1] = features[s : s + n1].T
        if cnt > f_half:
            xt[C:P, : cnt - f_half] = features[s + f_half : e].T
        in_maps.append({
            "xt": xt,
            "invn": np.full((P, 1), 1.0 / max(cnt, 1), dtype=np.float32),
            "gcol": gcol,
            "bcol": bcol,
            "foldm": foldm,
        })
    return in_maps, bounds, f_half


def _assemble(results, bounds, f_half):
    out = np.empty((N, C), dtype=np.float32)
    for b in range(BATCH):
        s, e = int(bounds[b]), int(bounds[b + 1])
        cnt = e - s
        if cnt == 0:
            continue
        ot = results[b]["out"]
        n1 = min(cnt, f_half)
        out[s : s + n1] = ot[0:C, :n1].T
        if cnt > f_half:
            out[s + f_half : e] = ot[C:P, : cnt - f_half].T
    return out


def run_with_results(features, batch_indices, gamma, beta, **run_kwargs):
    in_maps, bounds, f_half = _prepare(features, batch_indices, gamma, beta)
    nc = _build_program(f_half)
    res = run_bass_kernel_spmd(nc, in_maps, core_ids=list(range(BATCH)),
                               **run_kwargs)
    return _assemble(res.results, bounds, f_half), res


def kernel(features, batch_indices, gamma, beta):
    out, _ = run_with_results(features, batch_indices, gamma, beta)
    return out



# revision 11
# speedup vs baseline: 1.0068x; 1.0068x over previous
"""Trainium2 Bass kernel for CustomMinkowskiLayerNorm (v2: fp16 SBUF cache).

Math (matches the jax reference):
    counts[b]  = #points with batch_indices == b           (clamped >= 1)
    mean[b,c]  = sum_{i in b} x[i,c] / counts[b]
    var[b,c]   = sum_{i in b} (x[i,c]-mean)^2 / counts[b]  (= E[x^2]-mean^2)
    out[i,c]   = (x[i,c]-mean[b_i,c]) / sqrt(var[b_i,c]+eps) * gamma[c] + beta[c]

Sharding: batch_indices is sorted and BATCH == n_cores == 8, so each core owns
exactly one batch segment -> all reductions are core-local, no collectives.
Host splits at segment boundaries, transposes each segment to channel-major
and zero-pads:  xt[p, f]: partition p<64 = channel p, points [0, F_HALF);
p>=64 = channel p-64, points [F_HALF, 2*F_HALF).

v2 design (vs v1 two-pass with partial SBUF cache + HBM re-read):
  * The whole segment is cached in SBUF as fp16 (31 x 0.5 MiB), so pass 2
    never touches HBM for input -> total HBM traffic = 31 MiB read + 31 MiB
    write per core.  fp16 quantization adds ~5e-4 median rel err (tolerance
    2e-2).
  * Loads ride two concurrent queues (sync HWDGE + gpsimd SWDGE).  The SWDGE
    tiles use the in-flight fp32->fp16 cast, landing directly in the fp16
    cache with zero engine compute.  Two queues sustain ~425 GB/s vs ~390
    for one.
  * Stats: DVE bn_stats for most tiles; a few tiles (B set) use ScalarE
    Copy/Square with accum_out so the DVE has slack for the aggregations.
    Per-partition raw (sum, sumsq) pieces are folded + count-normalized by
    PSUM-accumulated matmuls against host-scaled 0/1 fold matrices
    (tot[p] = sums[p%64] + sums[p%64+64], scaled n_part/cnt).
  * Stores split across both HWDGE rings (sync + scalar).
  * Small inputs ride the SWDGE ring so the load burst starts at t=0.
  * The last tile is loaded as 4x512 chunks so the final bn_stats is 0.7us,
    not 2.7us; aggregation is split so only ~12 groups aggregate after the
    last chunk lands.
"""

import os
import sys

for _p in ("/opt/trn_rl_repo", "/root/.axon_site/_ro/trn_rl_repo"):
    if os.path.isdir(_p) and _p not in sys.path:
        sys.path.append(_p)

from contextlib import ExitStack

import numpy as np

import concourse.bacc as bacc
import concourse.tile as tile
from concourse import mybir
from concourse._compat import with_exitstack
from concourse.bass_utils import run_bass_kernel_spmd

F32 = mybir.dt.float32
F16 = mybir.dt.float16

N = 1_000_000
C = 64
BATCH = 8
EPS = 1e-5

P = 128            # SBUF partitions
F_TILE = 2048      # free elems per tile -> [128, 2048] f32 = 1 MiB per DMA
BN_F = 512         # bn_stats free-dim max
LOAD_BUFS = 4      # rotating fp32 load slots (sync-loaded tiles only)
OUT_BUFS = 4       # rotating fp32 pass-2 output slots
NEWTON = 1         # rsqrt Newton refinements after ACT sqrt + DVE recip

_mult = mybir.AluOpType.mult
_add = mybir.AluOpType.add
_AF = mybir.ActivationFunctionType


def _tile_plan(nt: int):
    """Assign tiles to load paths / stats paths.

    Returns (g_set, b_set, tail_tiles):
      g_set: swdge cast-loaded tiles (fp16 direct; DVE bn_stats on fp16)
      b_set: ScalarE-stats tiles (Copy+accum sum, Square+accum sumsq)
      the last tile (nt-1) is chunk-loaded; always sync + DVE.
    """
    if nt < 10:
        return set(), set(), nt - 1
    g_set = {t for t in range(4, nt - 3, 4)}
    b_set = {t for t in range(2, nt - 3, 8) if t not in g_set}
    return g_set, b_set, nt - 1


def _make_body(f_half: int):
    nt = f_half // F_TILE
    g_set, b_set, last = _tile_plan(nt)
    n_b = len(b_set)

    # dve-stats tiles: everything not in b_set. Their bn_stats groups are
    # written in tile order; aggr1 covers groups of tiles < nt-3, aggr2 the
    # rest (the final ~3 tiles incl. the chunked last tile).
    dve_tiles = [t for t in range(nt) if t not in b_set]
    agg_split_tile = nt - 3 if nt >= 10 else 0
    grp_of = {}
    g = 0
    ga = 0
    for t in dve_tiles:
        grp_of[t] = g
        if t < agg_split_tile:
            ga = g + F_TILE // BN_F
        g += F_TILE // BN_F
    gtot = g

    @with_exitstack
    def _body(ctx: ExitStack, tc: tile.TileContext,
              out_ap, xt_ap, gcol_ap, bcol_ap,
              fold1_ap, fold2_ap, fold3_ap):
        nc = tc.nc
        ngrp = F_TILE // BN_F

        cache = ctx.enter_context(tc.tile_pool(name="cache", bufs=nt))
        lpool = ctx.enter_context(tc.tile_pool(name="lpool", bufs=LOAD_BUFS))
        opool = ctx.enter_context(tc.tile_pool(name="opool", bufs=OUT_BUFS))
        small = ctx.enter_context(tc.tile_pool(name="small", bufs=1))
        psum = ctx.enter_context(tc.tile_pool(name="psum", bufs=1, space="PSUM"))

        stats = small.tile([P, max(gtot, 1), 6], F32, tag="stats")
        accs = None
        psq = None
        if n_b:
            accs = small.tile([P, n_b, 2], F32, tag="accs")
            psq = psum.tile([P, F_TILE], F32, tag="psq")

        # G-tile cast-loads go first on the SWDGE ring so payload moves from
        # t=0; the small inputs follow (needed only mid/late kernel).
        cached = {}
        for t in sorted(g_set):
            sl = slice(t * F_TILE, (t + 1) * F_TILE)
            xt16 = cache.tile([P, F_TILE], F16, tag="c")
            cached[t] = xt16
            nc.gpsimd.dma_start(out=xt16, in_=xt_ap[:, sl])

        gcol_sb = small.tile([P, 1], F32, tag="gcol")
        bcol_sb = small.tile([P, 1], F32, tag="bcol")
        fold1_sb = small.tile([P, P], F32, tag="fold1")
        fold2_sb = small.tile([P, P], F32, tag="fold2")
        fold3_sb = None
        nc.gpsimd.dma_start(out=fold1_sb, in_=fold1_ap)
        nc.gpsimd.dma_start(out=fold2_sb, in_=fold2_ap)
        if n_b:
            fold3_sb = small.tile([P, P], F32, tag="fold3")
            nc.gpsimd.dma_start(out=fold3_sb, in_=fold3_ap)
        nc.gpsimd.dma_start(out=gcol_sb, in_=gcol_ap)
        nc.gpsimd.dma_start(out=bcol_sb, in_=bcol_ap)

        # Pre-load the ACT sqrt table so the stats tail doesn't pay
        # ACT_TABLE_LOAD.
        warm = small.tile([P, 1], F32, tag="warm")
        nc.vector.memset(warm, 1.0)
        nc.scalar.activation(out=warm, in_=warm, func=_AF.Sqrt)

        ptot = psum.tile([P, 2], F32, tag="ptot")
        mm_total = (1 if ga > 0 else 0) + (1 if n_b else 0) + 1
        mm_done = 0

        def fold_mm(cols, fold_sb):
            nonlocal mm_done
            nc.tensor.matmul(out=ptot, lhsT=fold_sb, rhs=cols,
                             start=(mm_done == 0), stop=(mm_done == mm_total - 1))
            mm_done += 1

        def raw_cols(mv, tag):
            # [mean, var] over n elems -> cols [mean, var+mean^2]; the n/cnt
            # scale lives in the fold matrix.
            cols = small.tile([P, 2], F32, tag=tag)
            nc.vector.tensor_mul(out=cols[:, 1:2], in0=mv[:, 0:1], in1=mv[:, 0:1])
            nc.vector.tensor_add(out=cols[:, 1:2], in0=cols[:, 1:2], in1=mv[:, 1:2])
            nc.vector.tensor_copy(out=cols[:, 0:1], in_=mv[:, 0:1])
            return cols

        # ---- pass 1: stream tiles on two queues; stats on DVE / ScalarE ----
        b_idx = {t: i for i, t in enumerate(sorted(b_set))}
        aggr1_emitted = False
        mva = small.tile([P, 2], F32, tag="mva")
        mvb = small.tile([P, 2], F32, tag="mvb")

        for t in range(nt):
            sl = slice(t * F_TILE, (t + 1) * F_TILE)
            if t in g_set:
                # SWDGE cast-load already issued up top; just the stats here.
                xt16 = cached[t]
                for j in range(ngrp):
                    nc.vector.bn_stats(
                        out=stats[:, grp_of[t] + j, :],
                        in_=xt16[:, j * BN_F:(j + 1) * BN_F])
            elif t == last:
                # Final tile in 4 chunks: stats pipeline with the DMAs and
                # only ~0.7us of work follows the last chunk.
                xt16 = cache.tile([P, F_TILE], F16, tag="c")
                cached[t] = xt16
                xt32 = lpool.tile([P, F_TILE], F32, tag="l")
                for j in range(ngrp):
                    cs = slice(t * F_TILE + j * BN_F, t * F_TILE + (j + 1) * BN_F)
                    nc.sync.dma_start(out=xt32[:, j * BN_F:(j + 1) * BN_F],
                                      in_=xt_ap[:, cs])
                for j in range(ngrp):
                    nc.vector.bn_stats(
                        out=stats[:, grp_of[t] + j, :],
                        in_=xt32[:, j * BN_F:(j + 1) * BN_F])
                    nc.scalar.activation(out=xt16[:, j * BN_F:(j + 1) * BN_F],
                                         in_=xt32[:, j * BN_F:(j + 1) * BN_F],
                                         func=_AF.Copy)
            else:
                xt16 = cache.tile([P, F_TILE], F16, tag="c")
                cached[t] = xt16
                xt32 = lpool.tile([P, F_TILE], F32, tag="l")
                nc.sync.dma_start(out=xt32, in_=xt_ap[:, sl])
                if t in b_set:
                    bi = b_idx[t]
                    nc.scalar.activation(out=xt16, in_=xt32, func=_AF.Copy,
                                         accum_out=accs[:, bi, 0:1])
                    nc.scalar.activation(out=psq, in_=xt32, func=_AF.Square,
                                         accum_out=accs[:, bi, 1:2])
                else:
                    nc.scalar.activation(out=xt16, in_=xt32, func=_AF.Copy)
                    for j in range(ngrp):
                        nc.vector.bn_stats(
                            out=stats[:, grp_of[t] + j, :],
                            in_=xt32[:, j * BN_F:(j + 1) * BN_F])
            if t == agg_split_tile - 1 and ga > 0:
                # Early aggregation of everything seen so far (runs on DVE
                # while the tail tiles stream in).
                nc.vector.bn_aggr(out=mva, in_=stats[:, :ga, :])
                cols1 = raw_cols(mva, "cols1")
                fold_mm(cols1, fold1_sb)
                aggr1_emitted = True
                if n_b:
                    asums = small.tile([P, 2], F32, tag="asums")
                    acc_view = accs.rearrange("p t c -> p c t")
                    nc.vector.reduce_sum(out=asums, in_=acc_view,
                                         axis=mybir.AxisListType.X)
                    fold_mm(asums, fold3_sb)

        # ---- final aggregation (short: ~12 groups) ----
        if aggr1_emitted:
            nc.vector.bn_aggr(out=mvb, in_=stats[:, ga:, :])
        else:
            nc.vector.bn_aggr(out=mvb, in_=stats[:, :, :])
            if n_b:
                asums = small.tile([P, 2], F32, tag="asums")
                acc_view = accs.rearrange("p t c -> p c t")
                nc.vector.reduce_sum(out=asums, in_=acc_view,
                                     axis=mybir.AxisListType.X)
                fold_mm(asums, fold3_sb)
        cols2 = raw_cols(mvb, "cols2")
        fold_mm(cols2, fold2_sb)

        # ---- per-channel coefficients ----
        tot = small.tile([P, 2], F32, tag="tot")   # (mean, E[x^2]) per channel
        nc.vector.tensor_copy(out=tot, in_=ptot)
        var = small.tile([P, 1], F32, tag="var")
        nc.vector.tensor_mul(out=var, in0=tot[:, 0:1], in1=tot[:, 0:1])
        nc.vector.tensor_sub(out=var, in0=tot[:, 1:2], in1=var)
        v = small.tile([P, 1], F32, tag="v")
        nc.vector.tensor_scalar(out=v, in0=var, scalar1=0.0, scalar2=EPS,
                                op0=mybir.AluOpType.max, op1=_add)
        r = small.tile([P, 1], F32, tag="r")
        nc.scalar.activation(out=r, in_=v, func=_AF.Sqrt)
        nc.vector.reciprocal(out=r, in_=r)
        a = small.tile([P, 1], F32, tag="a")
        for _ in range(NEWTON):
            nc.vector.tensor_mul(out=a, in0=r, in1=r)
            nc.vector.tensor_mul(out=a, in0=a, in1=v)
            nc.vector.tensor_scalar(out=a, in0=a, scalar1=-0.5, scalar2=1.5,
                                    op0=_mult, op1=_add)
            nc.vector.tensor_mul(out=r, in0=r, in1=a)
        s_col = small.tile([P, 1], F32, tag="s_col")
        nc.vector.tensor_mul(out=s_col, in0=r, in1=gcol_sb)
        t_col = small.tile([P, 1], F32, tag="t_col")
        nc.vector.tensor_mul(out=t_col, in0=tot[:, 0:1], in1=s_col)
        nc.vector.tensor_sub(out=t_col, in0=bcol_sb, in1=t_col)

        # ---- pass 2: out32 = cache16*s + t, stores split on both rings ----
        for i in range(nt):
            sl = slice(i * F_TILE, (i + 1) * F_TILE)
            o32 = opool.tile([P, F_TILE], F32, tag="o")
            nc.vector.tensor_scalar(out=o32, in0=cached[i],
                                    scalar1=s_col[:, 0:1], scalar2=t_col[:, 0:1],
                                    op0=_mult, op1=_add)
            eng = nc.sync if (i % 2 == 0) else nc.scalar
            eng.dma_start(out=out_ap[:, sl], in_=o32)

    return _body


_NC_CACHE = {}


def _build_program(f_half: int):
    if f_half in _NC_CACHE:
        return _NC_CACHE[f_half]
    nc = bacc.Bacc("TRN2", target_bir_lowering=False, debug=False,
                   num_devices=BATCH)
    xt = nc.dram_tensor("xt", [P, f_half], F32, kind="ExternalInput").ap()
    gcol = nc.dram_tensor("gcol", [P, 1], F32, kind="ExternalInput").ap()
    bcol = nc.dram_tensor("bcol", [P, 1], F32, kind="ExternalInput").ap()
    fold1 = nc.dram_tensor("fold1", [P, P], F32, kind="ExternalInput").ap()
    fold2 = nc.dram_tensor("fold2", [P, P], F32, kind="ExternalInput").ap()
    fold3 = nc.dram_tensor("fold3", [P, P], F32, kind="ExternalInput").ap()
    out = nc.dram_tensor("out", [P, f_half], F32, kind="ExternalOutput").ap()
    with tile.TileContext(nc) as tc:
        _make_body(f_half)(tc, out, xt, gcol, bcol, fold1, fold2, fold3)
    nc.compile()
    _NC_CACHE[f_half] = nc
    return nc


def _part_elem_counts(f_half: int):
    """Elements (per partition) covered by aggr1/aggr2 vs the B (ACT) tiles."""
    nt = f_half // F_TILE
    g_set, b_set, _last = _tile_plan(nt)
    agg_split_tile = nt - 3 if nt >= 10 else 0
    n1 = sum(F_TILE for t in range(nt) if t not in b_set and t < agg_split_tile)
    n2 = sum(F_TILE for t in range(nt) if t not in b_set and t >= agg_split_tile)
    return n1, n2


def _prepare(features, batch_indices, gamma, beta):
    features = np.asarray(features, dtype=np.float32)
    batch_indices = np.asarray(batch_indices, dtype=np.int32)
    gamma = np.asarray(gamma, dtype=np.float32)
    beta = np.asarray(beta, dtype=np.float32)

    bounds = np.searchsorted(batch_indices, np.arange(BATCH + 1), side="left")
    cnts = np.diff(bounds)
    # fixed SPMD shape: half-row length, padded to a multiple of F_TILE
    f_half = max(int(-(-int(cnts.max()) // 2 // F_TILE) * F_TILE), F_TILE)
    n1, n2 = _part_elem_counts(f_half)

    gcol = np.concatenate([gamma, gamma]).reshape(P, 1).astype(np.float32)
    bcol = np.concatenate([beta, beta]).reshape(P, 1).astype(np.float32)
    k = np.arange(P)
    foldm = (k[:, None] % C == k[None, :] % C).astype(np.float32)

    in_maps = []
    for b in range(BATCH):
        s, e = int(bounds[b]), int(bounds[b + 1])
        cnt = e - s
        xt = np.zeros((P, f_half), dtype=np.float32)
        npts1 = min(cnt, f_half)
        if npts1 > 0:
            xt[0:C, :npts1] = features[s : s + npts1].T
        if cnt > f_half:
            xt[C:P, : cnt - f_half] = features[s + f_half : e].T
        inv = 1.0 / max(cnt, 1)
        in_maps.append({
            "xt": xt,
            "gcol": gcol,
            "bcol": bcol,
            "fold1": (foldm * (n1 * inv)).astype(np.float32),
            "fold2": (foldm * (n2 * inv)).astype(np.float32),
            "fold3": (foldm * inv).astype(np.float32),
        })
    return in_maps, bounds, f_half


def _assemble(results, bounds, f_half):
    out = np.empty((N, C), dtype=np.float32)
    for b in range(BATCH):
        s, e = int(bounds[b]), int(bounds[b + 1])
        cnt = e - s
        if cnt == 0:
            continue
        ot = results[b]["out"]
        npts1 = min(cnt, f_half)
        out[s : s + npts1] = ot[0:C, :npts1].T
        if cnt > f_half:
            out[s + f_half : e] = ot[C:P, : cnt - f_half].T
    return out


def run_with_results(features, batch_indices, gamma, beta, **run_kwargs):
    in_maps, bounds, f_half = _prepare(features, batch_indices, gamma, beta)
    nc = _build_program(f_half)
    res = run_bass_kernel_spmd(nc, in_maps, core_ids=list(range(BATCH)),
                               **run_kwargs)
    return _assemble(res.results, bounds, f_half), res


def kernel(features, batch_indices, gamma, beta):
    out, _ = run_with_results(features, batch_indices, gamma, beta)
    return out


# revision 15
# speedup vs baseline: 1.0562x; 1.0491x over previous
"""Trainium2 Bass kernel for CustomMinkowskiLayerNorm (v3: fp16 SBUF cache).

Math (matches the jax reference):
    counts[b]  = #points with batch_indices == b           (clamped >= 1)
    mean[b,c]  = sum_{i in b} x[i,c] / counts[b]
    var[b,c]   = sum_{i in b} (x[i,c]-mean)^2 / counts[b]  (= E[x^2]-mean^2)
    out[i,c]   = (x[i,c]-mean[b_i,c]) / sqrt(var[b_i,c]+eps) * gamma[c] + beta[c]

Sharding: batch_indices is sorted and BATCH == n_cores == 8, so each core owns
exactly one batch segment -> all reductions are core-local, no collectives.
Host splits at segment boundaries, transposes each segment to channel-major
and zero-pads:  xt[p, f]: partition p<64 = channel p, points [0, F_HALF);
p>=64 = channel p-64, points [F_HALF, 2*F_HALF).

Design:
  * The whole segment is cached in SBUF as fp16 (~15.5 MiB), so pass 2 never
    re-reads HBM: total traffic = 31 MiB in + 31 MiB out per core.  fp16
    quantization costs ~2e-4 median rel err (tolerance 2e-2).
  * Loads / stores each split across the two HWDGE rings (sync + scalar).
    SWDGE carries only the small inputs: its payload throughput (and
    especially the in-flight-cast path) measured far too slow in v2.
  * Stats: DVE bn_stats for most tiles; B-tiles use ScalarE Copy/Square with
    accum_out so the DVE has slack for the bn_aggr aggregations.  Raw
    per-partition (sum, sumsq) pieces are folded across partition halves and
    count-normalized by PSUM-accumulated matmuls against host-scaled 0/1
    fold matrices (tot[p] = sums[p%64]+sums[p%64+64], scale n_part/cnt).
  * The last tile is loaded as 4x512 chunks so only ~0.7us of bn_stats
    remains after the final chunk lands; aggr2 then covers only ~16 groups.
  * Coefficient tail is 4 ops: negvar = mean^2*1 - E[x^2] (one tensor_scalar
    with scalar ops), r = Sqrt(-negvar + eps) on ACT via scale=-1/bias=eps,
    s = gamma / r (DVE divide), t_neg = mean*s - beta (one tensor_scalar);
    pass 2 applies x*s - t_neg.
"""

import os
import sys

for _p in ("/opt/trn_rl_repo", "/root/.axon_site/_ro/trn_rl_repo"):
    if os.path.isdir(_p) and _p not in sys.path:
        sys.path.append(_p)

from contextlib import ExitStack

import numpy as np

import concourse.bacc as bacc
import concourse.tile as tile
from concourse import mybir
from concourse._compat import with_exitstack
from concourse.bass_utils import run_bass_kernel_spmd

F32 = mybir.dt.float32
F16 = mybir.dt.float16

N = 1_000_000
C = 64
BATCH = 8
EPS = 1e-5

P = 128            # SBUF partitions
F_TILE = 2048      # free elems per tile -> [128, 2048] f32 = 1 MiB per DMA
BN_F = 512         # bn_stats free-dim max
LOAD_BUFS = 5      # rotating fp32 load slots
OUT_BUFS = 4       # rotating fp32 pass-2 output slots
SCALAR_LOAD_EVERY = 4   # every 4th load issued on the scalar HWDGE ring
SWDGE_STORES = (1, 9)   # tiles stored via the SWDGE ring (3rd write queue)

_mult = mybir.AluOpType.mult
_add = mybir.AluOpType.add
_sub = mybir.AluOpType.subtract
_AF = mybir.ActivationFunctionType


def _tile_plan(nt: int):
    """b_set: ScalarE-stats tiles; agg_split: aggr1 covers DVE tiles below."""
    if nt < 10:
        return set(), 0
    b_set = {t for t in range(2, nt - 4, 8)} | {nt - 5}
    return b_set, nt - 5


def _make_body(f_half: int):
    nt = f_half // F_TILE
    last = nt - 1
    b_set, agg_split_tile = _tile_plan(nt)
    n_b = len(b_set)

    dve_tiles = [t for t in range(nt) if t not in b_set]
    grp_of = {}
    g = 0
    ga = 0
    for t in dve_tiles:
        grp_of[t] = g
        if t < agg_split_tile:
            ga = g + F_TILE // BN_F
        g += F_TILE // BN_F
    gtot = g

    @with_exitstack
    def _body(ctx: ExitStack, tc: tile.TileContext,
              out_ap, xt_ap, gcol_ap, bcol_ap,
              fold1_ap, fold2_ap, fold3_ap):
        nc = tc.nc
        ngrp = F_TILE // BN_F

        cache = ctx.enter_context(tc.tile_pool(name="cache", bufs=nt))
        lpool = ctx.enter_context(tc.tile_pool(name="lpool", bufs=LOAD_BUFS))
        opool = ctx.enter_context(tc.tile_pool(name="opool", bufs=OUT_BUFS))
        small = ctx.enter_context(tc.tile_pool(name="small", bufs=1))
        psum = ctx.enter_context(tc.tile_pool(name="psum", bufs=1, space="PSUM"))

        stats = small.tile([P, max(gtot, 1), 6], F32, tag="stats")
        accs = None
        psq = None
        if n_b:
            accs = small.tile([P, n_b, 2], F32, tag="accs")
            psq = psum.tile([P, F_TILE], F32, tag="psq")

        # Small inputs ride the SWDGE ring; they land well before first use
        # and never delay the HWDGE load burst.
        gcol_sb = small.tile([P, 1], F32, tag="gcol")
        bcol_sb = small.tile([P, 1], F32, tag="bcol")
        fold1_sb = small.tile([P, P], F32, tag="fold1")
        fold2_sb = small.tile([P, P], F32, tag="fold2")
        fold3_sb = None
        nc.gpsimd.dma_start(out=fold1_sb, in_=fold1_ap)
        nc.gpsimd.dma_start(out=fold2_sb, in_=fold2_ap)
        if n_b:
            fold3_sb = small.tile([P, P], F32, tag="fold3")
            nc.gpsimd.dma_start(out=fold3_sb, in_=fold3_ap)
        nc.gpsimd.dma_start(out=gcol_sb, in_=gcol_ap)
        nc.gpsimd.dma_start(out=bcol_sb, in_=bcol_ap)

        # Pre-load the ACT sqrt table so the tail doesn't pay ACT_TABLE_LOAD;
        # eps lives in a tiny tile (no const AP registered for 1e-5).
        warm = small.tile([P, 1], F32, tag="warm")
        nc.vector.memset(warm, 1.0)
        eps_sb = small.tile([P, 1], F32, tag="eps")
        nc.vector.memset(eps_sb, EPS)
        nc.scalar.activation(out=warm, in_=warm, func=_AF.Sqrt)

        ptot = psum.tile([P, 2], F32, tag="ptot")
        mm_total = (1 if ga > 0 else 0) + (1 if n_b else 0) + 1
        mm_done = 0

        def fold_mm(cols, fold_sb):
            nonlocal mm_done
            nc.tensor.matmul(out=ptot, lhsT=fold_sb, rhs=cols,
                             start=(mm_done == 0), stop=(mm_done == mm_total - 1))
            mm_done += 1

        def raw_cols(mv, tag):
            # [mean, var] over n elems -> cols [mean, var+mean^2]; the n/cnt
            # scale lives in the fold matrix.
            cols = small.tile([P, 2], F32, tag=tag)
            nc.vector.tensor_mul(out=cols[:, 1:2], in0=mv[:, 0:1], in1=mv[:, 0:1])
            nc.vector.tensor_add(out=cols[:, 1:2], in0=cols[:, 1:2], in1=mv[:, 1:2])
            nc.vector.tensor_copy(out=cols[:, 0:1], in_=mv[:, 0:1])
            return cols

        # ---- pass 1: stream tiles on both HWDGE rings ----
        cached = {}
        b_idx = {t: i for i, t in enumerate(sorted(b_set))}
        mva = small.tile([P, 2], F32, tag="mva")
        mvb = small.tile([P, 2], F32, tag="mvb")
        n_sc = 0

        for t in range(nt):
            sl = slice(t * F_TILE, (t + 1) * F_TILE)
            xt16 = cache.tile([P, F_TILE], F16, tag="c")
            cached[t] = xt16
            xt32 = lpool.tile([P, F_TILE], F32, tag="l")
            if t == last:
                # Final tile in 4 chunks: bn_stats pipelines with the chunk
                # DMAs; only ~0.7us of stats follows the last landing.
                for j in range(ngrp):
                    cs = slice(t * F_TILE + j * BN_F, t * F_TILE + (j + 1) * BN_F)
                    nc.sync.dma_start(out=xt32[:, j * BN_F:(j + 1) * BN_F],
                                      in_=xt_ap[:, cs])
                for j in range(ngrp):
                    nc.vector.bn_stats(
                        out=stats[:, grp_of[t] + j, :],
                        in_=xt32[:, j * BN_F:(j + 1) * BN_F])
                    nc.scalar.activation(out=xt16[:, j * BN_F:(j + 1) * BN_F],
                                         in_=xt32[:, j * BN_F:(j + 1) * BN_F],
                                         func=_AF.Copy)
            else:
                if t % SCALAR_LOAD_EVERY == SCALAR_LOAD_EVERY - 1:
                    nc.scalar.dma_start(out=xt32, in_=xt_ap[:, sl])
                    n_sc += 1
                else:
                    nc.sync.dma_start(out=xt32, in_=xt_ap[:, sl])
                if t in b_set:
                    bi = b_idx[t]
                    nc.scalar.activation(out=xt16, in_=xt32, func=_AF.Copy,
                                         accum_out=accs[:, bi, 0:1])
                    nc.scalar.activation(out=psq, in_=xt32, func=_AF.Square,
                                         accum_out=accs[:, bi, 1:2])
                else:
                    nc.scalar.activation(out=xt16, in_=xt32, func=_AF.Copy)
                    for j in range(ngrp):
                        nc.vector.bn_stats(
                            out=stats[:, grp_of[t] + j, :],
                            in_=xt32[:, j * BN_F:(j + 1) * BN_F])
            if t == agg_split_tile - 1 and ga > 0:
                # Early aggregation (DVE) + early folds while the last few
                # tiles stream in.
                nc.vector.bn_aggr(out=mva, in_=stats[:, :ga, :])
                cols1 = raw_cols(mva, "cols1")
                fold_mm(cols1, fold1_sb)

        # B accumulators -> raw sums -> fold (last B tile is nt-5, so this is
        # off the critical path by the time the final chunks land).
        if n_b:
            asums = small.tile([P, 2], F32, tag="asums")
            acc_view = accs.rearrange("p t c -> p c t")
            nc.vector.reduce_sum(out=asums, in_=acc_view,
                                 axis=mybir.AxisListType.X)
            fold_mm(asums, fold3_sb)

        # ---- final aggregation (short: last few tiles' groups) ----
        if ga > 0:
            nc.vector.bn_aggr(out=mvb, in_=stats[:, ga:, :])
        else:
            nc.vector.bn_aggr(out=mvb, in_=stats[:, :, :])
        cols2 = raw_cols(mvb, "cols2")
        fold_mm(cols2, fold2_sb)

        # ---- per-channel coefficients (4 ops + PSUM copy) ----
        tot = small.tile([P, 2], F32, tag="tot")   # (mean, E[x^2]) per channel
        nc.vector.tensor_copy(out=tot, in_=ptot)
        negv = small.tile([P, 1], F32, tag="negv")  # mean^2 - E[x^2]
        nc.vector.tensor_scalar(out=negv, in0=tot[:, 0:1],
                                scalar1=tot[:, 0:1], scalar2=tot[:, 1:2],
                                op0=_mult, op1=_sub)
        r = small.tile([P, 1], F32, tag="r")        # sqrt(var + eps)
        nc.scalar.activation(out=r, in_=negv, func=_AF.Sqrt,
                             scale=-1.0, bias=eps_sb[:, 0:1])
        nc.vector.reciprocal(out=r, in_=r)
        s_col = small.tile([P, 1], F32, tag="s_col")
        nc.vector.tensor_mul(out=s_col, in0=gcol_sb, in1=r)
        tneg = small.tile([P, 1], F32, tag="tneg")  # mean*s - beta
        nc.vector.tensor_scalar(out=tneg, in0=tot[:, 0:1],
                                scalar1=s_col[:, 0:1], scalar2=bcol_sb[:, 0:1],
                                op0=_mult, op1=_sub)

        # ---- pass 2: out32 = cache16*s - tneg; stores on all three rings ----
        for i in range(nt):
            sl = slice(i * F_TILE, (i + 1) * F_TILE)
            o32 = opool.tile([P, F_TILE], F32, tag="o")
            nc.vector.tensor_scalar(out=o32, in0=cached[i],
                                    scalar1=s_col[:, 0:1], scalar2=tneg[:, 0:1],
                                    op0=_mult, op1=_sub)
            if i in SWDGE_STORES:
                nc.gpsimd.dma_start(out=out_ap[:, sl], in_=o32)
            elif i % 2 == 0:
                nc.sync.dma_start(out=out_ap[:, sl], in_=o32)
            else:
                nc.scalar.dma_start(out=out_ap[:, sl], in_=o32)

    return _body


_NC_CACHE = {}


def _build_program(f_half: int):
    if f_half in _NC_CACHE:
        return _NC_CACHE[f_half]
    nc = bacc.Bacc("TRN2", target_bir_lowering=False, debug=False,
                   num_devices=BATCH)
    xt = nc.dram_tensor("xt", [P, f_half], F32, kind="ExternalInput").ap()
    gcol = nc.dram_tensor("gcol", [P, 1], F32, kind="ExternalInput").ap()
    bcol = nc.dram_tensor("bcol", [P, 1], F32, kind="ExternalInput").ap()
    fold1 = nc.dram_tensor("fold1", [P, P], F32, kind="ExternalInput").ap()
    fold2 = nc.dram_tensor("fold2", [P, P], F32, kind="ExternalInput").ap()
    fold3 = nc.dram_tensor("fold3", [P, P], F32, kind="ExternalInput").ap()
    out = nc.dram_tensor("out", [P, f_half], F32, kind="ExternalOutput").ap()
    with tile.TileContext(nc) as tc:
        _make_body(f_half)(tc, out, xt, gcol, bcol, fold1, fold2, fold3)
    nc.compile()
    _NC_CACHE[f_half] = nc
    return nc


def _part_elem_counts(f_half: int):
    """Per-partition element counts behind aggr1 / aggr2 (B tiles excluded)."""
    nt = f_half // F_TILE
    b_set, agg_split_tile = _tile_plan(nt)
    n1 = sum(F_TILE for t in range(nt) if t not in b_set and t < agg_split_tile)
    n2 = sum(F_TILE for t in range(nt) if t not in b_set and t >= agg_split_tile)
    return n1, n2


def _prepare(features, batch_indices, gamma, beta):
    features = np.asarray(features, dtype=np.float32)
    batch_indices = np.asarray(batch_indices, dtype=np.int32)
    gamma = np.asarray(gamma, dtype=np.float32)
    beta = np.asarray(beta, dtype=np.float32)

    bounds = np.searchsorted(batch_indices, np.arange(BATCH + 1), side="left")
    cnts = np.diff(bounds)
    # fixed SPMD shape: half-row length, padded to a multiple of F_TILE
    f_half = max(int(-(-int(cnts.max()) // 2 // F_TILE) * F_TILE), F_TILE)
    n1, n2 = _part_elem_counts(f_half)

    gcol = np.concatenate([gamma, gamma]).reshape(P, 1).astype(np.float32)
    bcol = np.concatenate([beta, beta]).reshape(P, 1).astype(np.float32)
    k = np.arange(P)
    foldm = (k[:, None] % C == k[None, :] % C).astype(np.float32)

    in_maps = []
    for b in range(BATCH):
        s, e = int(bounds[b]), int(bounds[b + 1])
        cnt = e - s
        xt = np.zeros((P, f_half), dtype=np.float32)
        npts1 = min(cnt, f_half)
        if npts1 > 0:
            xt[0:C, :npts1] = features[s : s + npts1].T
        if cnt > f_half:
            xt[C:P, : cnt - f_half] = features[s + f_half : e].T
        inv = 1.0 / max(cnt, 1)
        in_maps.append({
            "xt": xt,
            "gcol": gcol,
            "bcol": bcol,
            "fold1": (foldm * (n1 * inv)).astype(np.float32),
            "fold2": (foldm * (n2 * inv)).astype(np.float32),
            "fold3": (foldm * inv).astype(np.float32),
        })
    return in_maps, bounds, f_half


def _assemble(results, bounds, f_half):
    out = np.empty((N, C), dtype=np.float32)
    for b in range(BATCH):
        s, e = int(bounds[b]), int(bounds[b + 1])
        cnt = e - s
        if cnt == 0:
            continue
        ot = results[b]["out"]
        npts1 = min(cnt, f_half)
        out[s : s + npts1] = ot[0:C, :npts1].T
        if cnt > f_half:
            out[s + f_half : e] = ot[C:P, : cnt - f_half].T
    return out


def run_with_results(features, batch_indices, gamma, beta, **run_kwargs):
    in_maps, bounds, f_half = _prepare(features, batch_indices, gamma, beta)
    nc = _build_program(f_half)
    res = run_bass_kernel_spmd(nc, in_maps, core_ids=list(range(BATCH)),
                               **run_kwargs)
    return _assemble(res.results, bounds, f_half), res


def kernel(features, batch_indices, gamma, beta):
    out, _ = run_with_results(features, batch_indices, gamma, beta)
    return out
